# revision 6
# baseline (speedup 1.0000x reference)
"""Trainium2 Bass kernel for nn_ChaosSSMCore (selective diag-SSM).

Reference computation per (b, t):
    z, s, u, g = x @ {W_delta, W_select, W_in, W_gate}^T
    delta  = softplus(z)
    decay  = exp(-delta * exp(log_a))
    update = delta * sigmoid(s) * u
    states = scan: st = decay_t * st_{t-1} + update_t    (per (b, d) lane)
    out    = (states * silu(g)) @ W_out^T

Device mapping (8 cores, batch-sharded: 16 batches/core):
  * Host casts x to fp16; x arrives pre-transposed [d, t] so d (the
    contraction dim) lands on partitions with plain contiguous DMA.
  * 4 input projections as fp16 matmuls (W^T stationary, x^T moving),
    PSUM results in [e, t] layout -> time on the free axis for the scan.
  * ONE activation-table set (silu_and_others: tanh + silu + relu) for the
    whole kernel; per-chunk set swaps would cost ~2.7us each.
  * Engine split tuned from the profile (Vector was the bottleneck at 85%):
      ScalarE  : tz=tanh(z/2), rz=relu(z'), ts=tanh(s/2), gs=silu(g),
                 out-proj PSUM->SBUF copy               (5 passes)
      VectorE  : dec = 0.5 - 0.5*tz             = sigmoid(-z)    [TS 4x]
                 at  = tz & 0x7fff              = |tz|           [TS bitvec]
                 w1  = at + A1;  w2 = at + A2                    [TS 4x]
                 su  = (ts + 1) * u'                             [STT, PSUM]
                 upd = su * dd                                   [TT 2x]
                 2x tensor_tensor_scan (the recurrence)
      GPSIMD   : sqe = w1*w2;  dd = rz' + sqe;  y = states*silu(g)
  * softplus via the exact identity softplus(z) = relu(z) + ln2 - ln(1+|t|),
    t = tanh(z/2), with ln2 - ln(1+|t|) ~= E1*(|t|+A1)*(|t|+A2) (minimax
    quadratic in factored form, |err| < 3.5e-3; the roots absorb the
    constant term). E1 folds into the host-side W_delta scale (relu path)
    and W_in scale (update product). |t| is exact: uint16-bitcast
    tensor_scalar AND clears the fp16 sign bit.
  * Output projection uses y-blocks as the stationary operand so the result
    lands in PSUM already in natural [t, e'] layout; ScalarE copies all 512
    tokens in one pass to SBUF fp16 and it is DMA'd out. Host upcasts.

log_a != 0 (never produced by setup_inputs, which inits log_a = zeros) falls
back to an exact numpy implementation since decay-via-tanh needs a == 1.
"""

import sys

for _p in ("/opt/trn_rl_repo", "/opt/pypackages"):
    if _p not in sys.path:
        sys.path.insert(0, _p)

import numpy as np

B, T, D = 128, 2048, 256
N_CORES = 8
NB = B // N_CORES          # batches per core
P = 128                    # SBUF partitions
CHUNK = 512                # tokens per pipeline chunk
NCHUNK = T // CHUNK
KT = D // P                # contraction k-tiles (2)
MT = D // P                # output e-tiles (2)

PZ, PS, PU, PG, PO = 0, 1, 2, 3, 4   # weight slots: delta, select, in, gate, out

# minimax quadratic fit of ln2 - ln(1+v) ~= E1*(v+A1)*(v+A2) on v in [0,1]
# (|err| < 3.5e-3); softplus(z) = relu(z) + that, with v = |tanh(z/2)|.
# A1/A2 = K -/+ sqrt(-E0) from the (v+K)^2 + E0 completed-square form.
E1 = 0.23902059723734254
_K = -1.9355823232625622
_A = 0.9278528261037748  # sqrt(0.8609108668505208)
A1 = _K - _A
A2 = _K + _A


def build_bass(nb=NB):
    from contextlib import ExitStack

    import concourse.bacc as bacc
    import concourse.mybir as mybir
    import concourse.tile as tile

    f16 = mybir.dt.float16
    f32 = mybir.dt.float32
    u16 = mybir.dt.uint16
    ALU = mybir.AluOpType
    ACT = mybir.ActivationFunctionType

    nc = bacc.Bacc("TRN2", target_bir_lowering=False)

    ntok = nb * T
    # x arrives host-transposed: [batch, d, t] so the kernel loads x^T tiles
    # (d on partitions) with plain contiguous DMA.
    x_t = nc.dram_tensor("x", [nb, D, T], f16, kind="ExternalInput").ap()
    w_t = nc.dram_tensor("w", [P, 5, KT, D], f16, kind="ExternalInput").ap()
    out_t = nc.dram_tensor("out", [ntok, D], f16, kind="ExternalOutput").ap()

    with tile.TileContext(nc) as tc:
        with ExitStack() as ctx:
            singles = ctx.enter_context(tc.tile_pool(name="singles", bufs=1))
            xt_pool = ctx.enter_context(tc.tile_pool(name="xtp", bufs=6))
            sb = ctx.enter_context(tc.tile_pool(name="sb", bufs=4))
            osb_pool = ctx.enter_context(tc.tile_pool(name="osb", bufs=4))
            psum = ctx.enter_context(tc.tile_pool(name="psum", bufs=1, space="PSUM"))

            w_sb = singles.tile([P, 5, KT, D], f16)
            nc.scalar.dma_start(out=w_sb, in_=w_t)

            for b in range(nb):
                prev_states = None
                for c in range(NCHUNK):
                    row0 = b * T + c * CHUNK

                    # ---- load x^T tiles (host pre-transposed) ----
                    xt = [
                        xt_pool.tile([P, CHUNK], f16, tag=f"xt{k}", name=f"xt{k}")
                        for k in range(KT)
                    ]
                    for k in range(KT):
                        nc.sync.dma_start(
                            out=xt[k],
                            in_=x_t[
                                b,
                                k * P : (k + 1) * P,
                                c * CHUNK : (c + 1) * CHUNK,
                            ],
                        )

                    # ---- projections: psum[e_m, t] ----
                    # 2 rotating psum buffers (4 banks) for the 4 projections;
                    # issue order Z, S, G, U so each buffer's previous tenant
                    # has early consumers (Z: tz+rz, S: ts) by reuse time.
                    def proj(pi):
                        ps = psum.tile(
                            [P, MT, CHUNK], f32, tag="pp", bufs=2, name=f"pp{pi}"
                        )
                        for m in range(MT):
                            for k in range(KT):
                                nc.tensor.matmul(
                                    ps[:, m, :],
                                    w_sb[:, pi, k, m * P : (m + 1) * P],
                                    xt[k],
                                    start=(k == 0),
                                    stop=(k == KT - 1),
                                )
                        return ps

                    tz = sb.tile([P, MT, CHUNK], f16, tag="tz")
                    rz = sb.tile([P, MT, CHUNK], f16, tag="rz")
                    tsl = sb.tile([P, MT, CHUNK], f16, tag="tsl")
                    gs = sb.tile([P, MT, CHUNK], f16, tag="gs")

                    pz = proj(PZ)
                    # z' = z/E1 (host-scaled W_delta): tz = tanh(z/2) exactly,
                    # rz = relu(z)/E1.
                    nc.scalar.activation(
                        out=tz, in_=pz, func=ACT.Tanh, scale=0.5 * E1
                    )
                    nc.scalar.activation(out=rz, in_=pz, func=ACT.Relu)

                    psl = proj(PS)
                    nc.scalar.activation(out=tsl, in_=psl, func=ACT.Tanh, scale=0.5)

                    # ---- VectorE: decay + softplus factor pieces ----
                    dec = sb.tile([P, MT, CHUNK], f16, tag="dec")
                    at = sb.tile([P, MT, CHUNK], f16, tag="at")
                    w1 = sb.tile([P, MT, CHUNK], f16, tag="w1")
                    w2 = sb.tile([P, MT, CHUNK], f16, tag="w2")
                    sqe = sb.tile([P, MT, CHUNK], f16, tag="sqe")
                    dd = sb.tile([P, MT, CHUNK], f16, tag="dd")
                    su = sb.tile([P, MT, CHUNK], f16, tag="su")
                    upd = sb.tile([P, MT, CHUNK], f16, tag="upd")
                    states = sb.tile([P, MT, CHUNK], f16, tag="states")
                    # decay = 0.5 - 0.5*tz = sigmoid(-z)
                    nc.vector.tensor_scalar(
                        out=dec, in0=tz, scalar1=-1.0, scalar2=-0.5,
                        op0=ALU.add, op1=ALU.mult,
                    )
                    # at = |tz| (clear fp16 sign bit; exact)
                    nc.vector.tensor_scalar(
                        out=at.bitcast(u16), in0=tz.bitcast(u16),
                        scalar1=0x7FFF, scalar2=None, op0=ALU.bitwise_and,
                    )
                    nc.vector.tensor_scalar(
                        out=w1, in0=at, scalar1=A1, scalar2=None, op0=ALU.add
                    )
                    nc.vector.tensor_scalar(
                        out=w2, in0=at, scalar1=A2, scalar2=None, op0=ALU.add
                    )
                    # ---- GPSIMD: sqe = w1*w2 ((|t|+K)^2+E0 in factored form),
                    # dd = rz + sqe = delta/E1
                    nc.gpsimd.tensor_mul(sqe, w1, w2)
                    nc.gpsimd.tensor_add(dd, rz, sqe)

                    pg = proj(PG)
                    nc.scalar.activation(out=gs, in_=pg, func=ACT.Silu)
                    pu = proj(PU)

                    # su = (ts + 1) * u'  (u' = 0.5*E1*u via host-scaled W_in)
                    nc.vector.scalar_tensor_tensor(
                        out=su, in0=tsl, scalar=1.0, in1=pu,
                        op0=ALU.add, op1=ALU.mult,
                    )
                    # upd = su * dd = delta * sigmoid(s) * u
                    nc.vector.tensor_mul(upd, su, dd)

                    for m in range(MT):
                        init = (
                            0.0
                            if prev_states is None
                            else prev_states[:, m, CHUNK - 1 : CHUNK]
                        )
                        nc.vector.tensor_tensor_scan(
                            out=states[:, m, :],
                            data0=dec[:, m, :],
                            data1=upd[:, m, :],
                            initial=init,
                            op0=ALU.mult,
                            op1=ALU.add,
                        )
                    prev_states = states

                    # ---- GPSIMD: y = states * silu(g) ----
                    y = sb.tile([P, MT, CHUNK], f16, tag="y")
                    nc.gpsimd.tensor_mul(y, states, gs)

                    # ---- out projection: y blocks stationary -> [t, e'] ----
                    po = psum.tile([P, 4, D], f32, tag="po", bufs=2)
                    for tt in range(CHUNK // P):
                        for k in range(KT):
                            nc.tensor.matmul(
                                po[:, tt, :],
                                y[:, k, tt * P : (tt + 1) * P],
                                w_sb[:, PO, k, :],
                                start=(k == 0),
                                stop=(k == KT - 1),
                            )
                    osb = osb_pool.tile([P, 4, D], f16, tag="osb")
                    nc.scalar.activation(out=osb, in_=po, func=ACT.Copy)
                    nc.sync.dma_start(
                        out=out_t[row0 : row0 + CHUNK, :].rearrange(
                            "(j p) d -> p j d", p=P
                        ),
                        in_=osb,
                    )
    nc.compile()
    return nc


def _pack_weight(w):
    # lhsT layout: [d_within_k (partition), k, e] with lhsT[dd, k, e] = W[e, 128k+dd]
    return (
        np.ascontiguousarray(np.asarray(w, np.float32).T)
        .reshape(KT, P, D)
        .transpose(1, 0, 2)
        .astype(np.float16)
    )


def prepare_inputs(x, W_in, W_select, W_gate, W_out, W_delta, log_a):
    x16 = (
        np.ascontiguousarray(np.asarray(x, np.float32))
        .astype(np.float16)
        .reshape(N_CORES, NB, T, D)
        .transpose(0, 1, 3, 2)  # -> [core, batch, d, t]
    )
    x16 = np.ascontiguousarray(x16)
    # W_delta scaled by 1/E1 (softplus quadratic leading-coeff fold);
    # W_in scaled by 0.5*E1 (sigmoid affine + that fold's inverse:
    # update = (delta/E1)*(1+tanh(s/2)) * u' with u' = 0.5*E1*u)
    w_delta_scaled = np.asarray(W_delta, np.float32) / E1
    w_in_scaled = np.asarray(W_in, np.float32) * (0.5 * E1)
    w_pack = np.ascontiguousarray(
        np.stack(
            [
                _pack_weight(w)
                for w in (w_delta_scaled, W_select, w_in_scaled, W_gate, W_out)
            ],
            axis=1,
        )
    )  # [P, 5, KT, D]
    return [{"x": x16[c], "w": w_pack} for c in range(N_CORES)]


def _numpy_fallback(x, W_in, W_select, W_gate, W_out, W_delta, log_a):
    # exact reference math; only used when log_a != 0 (setup_inputs never does)
    x = np.asarray(x, np.float32)
    z = x @ np.asarray(W_delta, np.float32).T
    delta = np.logaddexp(0.0, z)
    decay = np.exp(-delta * np.exp(np.asarray(log_a, np.float32)))
    u = x @ np.asarray(W_in, np.float32).T
    s = x @ np.asarray(W_select, np.float32).T
    upd = delta * (1.0 / (1.0 + np.exp(-s))) * u
    states = np.empty_like(upd)
    st = np.zeros((x.shape[0], x.shape[2]), np.float32)
    for t in range(x.shape[1]):
        st = decay[:, t] * st + upd[:, t]
        states[:, t] = st
    g = x @ np.asarray(W_gate, np.float32).T
    y = states * (g / (1.0 + np.exp(-g)))
    return y @ np.asarray(W_out, np.float32).T


_CACHE = {}


def run_on_hw(inputs, trace=False):
    from concourse.bass_utils import run_bass_kernel_spmd

    if "nc" not in _CACHE:
        _CACHE["nc"] = build_bass()
    nc = _CACHE["nc"]
    in_maps = prepare_inputs(**inputs)
    res = run_bass_kernel_spmd(nc, in_maps, core_ids=list(range(N_CORES)), trace=trace)
    out = (
        np.stack([res.results[c]["out"] for c in range(N_CORES)])
        .reshape(B, T, D)
        .astype(np.float32)
    )
    return out, res


def kernel(x, W_in, W_select, W_gate, W_out, W_delta, log_a):
    inputs = dict(
        x=x,
        W_in=W_in,
        W_select=W_select,
        W_gate=W_gate,
        W_out=W_out,
        W_delta=W_delta,
        log_a=log_a,
    )
    if not np.allclose(np.asarray(log_a, np.float32), 0.0):
        return _numpy_fallback(**inputs)
    out, _ = run_on_hw(inputs)
    return out


# revision 7
# speedup vs baseline: 1.0787x; 1.0787x over previous
"""Trainium2 Bass kernel for nn_ChaosSSMCore (selective diag-SSM).

Reference computation per (b, t):
    z, s, u, g = x @ {W_delta, W_select, W_in, W_gate}^T
    delta  = softplus(z)
    decay  = exp(-delta * exp(log_a))
    update = delta * sigmoid(s) * u
    states = scan: st = decay_t * st_{t-1} + update_t    (per (b, d) lane)
    out    = (states * silu(g)) @ W_out^T

Device mapping (8 cores, batch-sharded: 16 batches/core):
  * Host casts x to fp16; x arrives pre-transposed [d, t] so d (the
    contraction dim) lands on partitions with plain contiguous DMA.
  * 4 input projections as fp16 matmuls (W^T stationary, x^T moving),
    PSUM results in [e, t] layout -> time on the free axis for the scan.
  * ONE activation-table set (silu_and_others: tanh + silu + relu) for the
    whole kernel; per-chunk set swaps would cost ~2.7us each.
  * Engine split tuned from the profile (Vector was the bottleneck at 85%):
      ScalarE  : tz=tanh(z/2), rz=relu(z'), ts=tanh(s/2), gs=silu(g),
                 out-proj PSUM->SBUF copy               (5 passes)
      VectorE  : dec = 0.5 - 0.5*tz             = sigmoid(-z)    [TS 4x]
                 at  = tz & 0x7fff              = |tz|           [TS bitvec]
                 w1  = at + A1;  w2 = at + A2                    [TS 4x]
                 su  = (ts + 1) * u'                             [STT, PSUM]
                 upd = su * dd                                   [TT 2x]
                 2x tensor_tensor_scan (the recurrence)
      GPSIMD   : sqe = w1*w2;  dd = rz' + sqe;  y = states*silu(g)
  * softplus via the exact identity softplus(z) = relu(z) + ln2 - ln(1+|t|),
    t = tanh(z/2), with ln2 - ln(1+|t|) ~= E1*(|t|+A1)*(|t|+A2) (minimax
    quadratic in factored form, |err| < 3.5e-3; the roots absorb the
    constant term). E1 folds into the host-side W_delta scale (relu path)
    and W_in scale (update product). |t| is exact: uint16-bitcast
    tensor_scalar AND clears the fp16 sign bit.
  * Output projection uses y-blocks as the stationary operand so the result
    lands in PSUM already in natural [t, e'] layout; ScalarE copies all 512
    tokens in one pass to SBUF fp16 and it is DMA'd out. Host upcasts.

log_a != 0 (never produced by setup_inputs, which inits log_a = zeros) falls
back to an exact numpy implementation since decay-via-tanh needs a == 1.
"""

import sys

for _p in ("/opt/trn_rl_repo", "/opt/pypackages"):
    if _p not in sys.path:
        sys.path.insert(0, _p)

import numpy as np

B, T, D = 128, 2048, 256
N_CORES = 8
NB = B // N_CORES          # batches per core
P = 128                    # SBUF partitions
CHUNK = 512                # tokens per pipeline chunk
NCHUNK = T // CHUNK
KT = D // P                # contraction k-tiles (2)
MT = D // P                # output e-tiles (2)

PZ, PS, PU, PG, PO = 0, 1, 2, 3, 4   # weight slots: delta, select, in, gate, out

# minimax quadratic fit of ln2 - ln(1+v) ~= E1*(v+A1)*(v+A2) on v in [0,1]
# (|err| < 3.5e-3); softplus(z) = relu(z) + that, with v = |tanh(z/2)|.
# A1/A2 = K -/+ sqrt(-E0) from the (v+K)^2 + E0 completed-square form.
E1 = 0.23902059723734254
_K = -1.9355823232625622
_A = 0.9278528261037748  # sqrt(0.8609108668505208)
A1 = _K - _A
A2 = _K + _A


def build_bass(nb=NB):
    from contextlib import ExitStack

    import concourse.bacc as bacc
    import concourse.mybir as mybir
    import concourse.tile as tile

    f16 = mybir.dt.float16
    f32 = mybir.dt.float32
    u16 = mybir.dt.uint16
    ALU = mybir.AluOpType
    ACT = mybir.ActivationFunctionType

    nc = bacc.Bacc("TRN2", target_bir_lowering=False)

    ntok = nb * T
    # x arrives host-transposed: [batch, d, t] so the kernel loads x^T tiles
    # (d on partitions) with plain contiguous DMA.
    x_t = nc.dram_tensor("x", [nb, D, T], f16, kind="ExternalInput").ap()
    w_t = nc.dram_tensor("w", [P, 5, KT, D], f16, kind="ExternalInput").ap()
    out_t = nc.dram_tensor("out", [ntok, D], f16, kind="ExternalOutput").ap()

    with tile.TileContext(nc) as tc:
        with ExitStack() as ctx:
            singles = ctx.enter_context(tc.tile_pool(name="singles", bufs=1))
            xt_pool = ctx.enter_context(tc.tile_pool(name="xtp", bufs=6))
            sb = ctx.enter_context(tc.tile_pool(name="sb", bufs=4))
            osb_pool = ctx.enter_context(tc.tile_pool(name="osb", bufs=4))
            psum = ctx.enter_context(tc.tile_pool(name="psum", bufs=1, space="PSUM"))

            w_sb = singles.tile([P, 5, KT, D], f16)
            nc.scalar.dma_start(out=w_sb, in_=w_t)

            for b in range(nb):
                prev_states = None
                for c in range(NCHUNK):
                    row0 = b * T + c * CHUNK

                    # ---- load x^T tiles (host pre-transposed) ----
                    xt = [
                        xt_pool.tile([P, CHUNK], f16, tag=f"xt{k}", name=f"xt{k}")
                        for k in range(KT)
                    ]
                    for k in range(KT):
                        nc.sync.dma_start(
                            out=xt[k],
                            in_=x_t[
                                b,
                                k * P : (k + 1) * P,
                                c * CHUNK : (c + 1) * CHUNK,
                            ],
                        )

                    # ---- projections: psum[e_m, t] ----
                    # 2 rotating psum buffers (4 banks) for the 4 projections;
                    # issue order Z, S, G, U so each buffer's previous tenant
                    # has early consumers (Z: tz+rz, S: ts) by reuse time.
                    def proj(pi):
                        ps = psum.tile(
                            [P, MT, CHUNK], f32, tag="pp", bufs=2, name=f"pp{pi}"
                        )
                        for m in range(MT):
                            for k in range(KT):
                                nc.tensor.matmul(
                                    ps[:, m, :],
                                    w_sb[:, pi, k, m * P : (m + 1) * P],
                                    xt[k],
                                    start=(k == 0),
                                    stop=(k == KT - 1),
                                )
                        return ps

                    tz = sb.tile([P, MT, CHUNK], f16, tag="tz")
                    rz = sb.tile([P, MT, CHUNK], f16, tag="rz")
                    tsl = sb.tile([P, MT, CHUNK], f16, tag="tsl")
                    gs = sb.tile([P, MT, CHUNK], f16, tag="gs")

                    pz = proj(PZ)
                    # z' = z/E1 (host-scaled W_delta): tz = tanh(z/2) exactly,
                    # rz = relu(z)/E1.
                    nc.scalar.activation(
                        out=tz, in_=pz, func=ACT.Tanh, scale=0.5 * E1
                    )
                    nc.scalar.activation(out=rz, in_=pz, func=ACT.Relu)

                    psl = proj(PS)
                    nc.scalar.activation(out=tsl, in_=psl, func=ACT.Tanh, scale=0.5)

                    # ---- VectorE: decay + softplus factor pieces ----
                    dec = sb.tile([P, MT, CHUNK], f16, tag="dec")
                    at = sb.tile([P, MT, CHUNK], f16, tag="at")
                    w1 = sb.tile([P, MT, CHUNK], f16, tag="w1")
                    w2 = sb.tile([P, MT, CHUNK], f16, tag="w2")
                    sqe = sb.tile([P, MT, CHUNK], f16, tag="sqe")
                    dd = sb.tile([P, MT, CHUNK], f16, tag="dd")
                    su = sb.tile([P, MT, CHUNK], f16, tag="su")
                    upd = sb.tile([P, MT, CHUNK], f16, tag="upd")
                    states = sb.tile([P, MT, CHUNK], f16, tag="states")
                    # decay = 0.5 - 0.5*tz = sigmoid(-z)
                    nc.vector.tensor_scalar(
                        out=dec, in0=tz, scalar1=-1.0, scalar2=-0.5,
                        op0=ALU.add, op1=ALU.mult,
                    )
                    # at = |tz| (clear fp16 sign bit; exact)
                    nc.vector.tensor_scalar(
                        out=at.bitcast(u16), in0=tz.bitcast(u16),
                        scalar1=0x7FFF, scalar2=None, op0=ALU.bitwise_and,
                    )
                    nc.vector.tensor_scalar(
                        out=w1, in0=at, scalar1=A1, scalar2=None, op0=ALU.add
                    )
                    nc.vector.tensor_scalar(
                        out=w2, in0=at, scalar1=A2, scalar2=None, op0=ALU.add
                    )
                    # sqe = w1*w2 ((|t|+K)^2+E0 in factored form), dd = delta/E1.
                    # On VectorE: GPSIMD shares the SBUF port with the DVE, so
                    # gp work only overlaps 1-port DVE ops (scan/STT) -- cheap
                    # 2x TTs here beat "free" gp TTs that stall the DVE.
                    nc.vector.tensor_mul(sqe, w1, w2)
                    nc.vector.tensor_add(dd, rz, sqe)

                    pg = proj(PG)
                    nc.scalar.activation(out=gs, in_=pg, func=ACT.Silu)
                    pu = proj(PU)

                    # su = (ts + 1) * u'  (u' = 0.5*E1*u via host-scaled W_in)
                    nc.vector.scalar_tensor_tensor(
                        out=su, in0=tsl, scalar=1.0, in1=pu,
                        op0=ALU.add, op1=ALU.mult,
                    )
                    # upd = su * dd = delta * sigmoid(s) * u
                    nc.vector.tensor_mul(upd, su, dd)

                    for m in range(MT):
                        init = (
                            0.0
                            if prev_states is None
                            else prev_states[:, m, CHUNK - 1 : CHUNK]
                        )
                        nc.vector.tensor_tensor_scan(
                            out=states[:, m, :],
                            data0=dec[:, m, :],
                            data1=upd[:, m, :],
                            initial=init,
                            op0=ALU.mult,
                            op1=ALU.add,
                        )
                    prev_states = states

                    # ---- GPSIMD: y = states * silu(g) ----
                    y = sb.tile([P, MT, CHUNK], f16, tag="y")
                    nc.gpsimd.tensor_mul(y, states, gs)

                    # ---- out projection: y blocks stationary -> [t, e'] ----
                    po = psum.tile([P, 4, D], f32, tag="po", bufs=2)
                    for tt in range(CHUNK // P):
                        for k in range(KT):
                            nc.tensor.matmul(
                                po[:, tt, :],
                                y[:, k, tt * P : (tt + 1) * P],
                                w_sb[:, PO, k, :],
                                start=(k == 0),
                                stop=(k == KT - 1),
                            )
                    osb = osb_pool.tile([P, 4, D], f16, tag="osb")
                    nc.scalar.activation(out=osb, in_=po, func=ACT.Copy)
                    nc.sync.dma_start(
                        out=out_t[row0 : row0 + CHUNK, :].rearrange(
                            "(j p) d -> p j d", p=P
                        ),
                        in_=osb,
                    )
    nc.compile()
    return nc


def _pack_weight(w):
    # lhsT layout: [d_within_k (partition), k, e] with lhsT[dd, k, e] = W[e, 128k+dd]
    return (
        np.ascontiguousarray(np.asarray(w, np.float32).T)
        .reshape(KT, P, D)
        .transpose(1, 0, 2)
        .astype(np.float16)
    )


def prepare_inputs(x, W_in, W_select, W_gate, W_out, W_delta, log_a):
    x16 = (
        np.ascontiguousarray(np.asarray(x, np.float32))
        .astype(np.float16)
        .reshape(N_CORES, NB, T, D)
        .transpose(0, 1, 3, 2)  # -> [core, batch, d, t]
    )
    x16 = np.ascontiguousarray(x16)
    # W_delta scaled by 1/E1 (softplus quadratic leading-coeff fold);
    # W_in scaled by 0.5*E1 (sigmoid affine + that fold's inverse:
    # update = (delta/E1)*(1+tanh(s/2)) * u' with u' = 0.5*E1*u)
    w_delta_scaled = np.asarray(W_delta, np.float32) / E1
    w_in_scaled = np.asarray(W_in, np.float32) * (0.5 * E1)
    w_pack = np.ascontiguousarray(
        np.stack(
            [
                _pack_weight(w)
                for w in (w_delta_scaled, W_select, w_in_scaled, W_gate, W_out)
            ],
            axis=1,
        )
    )  # [P, 5, KT, D]
    return [{"x": x16[c], "w": w_pack} for c in range(N_CORES)]


def _numpy_fallback(x, W_in, W_select, W_gate, W_out, W_delta, log_a):
    # exact reference math; only used when log_a != 0 (setup_inputs never does)
    x = np.asarray(x, np.float32)
    z = x @ np.asarray(W_delta, np.float32).T
    delta = np.logaddexp(0.0, z)
    decay = np.exp(-delta * np.exp(np.asarray(log_a, np.float32)))
    u = x @ np.asarray(W_in, np.float32).T
    s = x @ np.asarray(W_select, np.float32).T
    upd = delta * (1.0 / (1.0 + np.exp(-s))) * u
    states = np.empty_like(upd)
    st = np.zeros((x.shape[0], x.shape[2]), np.float32)
    for t in range(x.shape[1]):
        st = decay[:, t] * st + upd[:, t]
        states[:, t] = st
    g = x @ np.asarray(W_gate, np.float32).T
    y = states * (g / (1.0 + np.exp(-g)))
    return y @ np.asarray(W_out, np.float32).T


_CACHE = {}


def run_on_hw(inputs, trace=False):
    from concourse.bass_utils import run_bass_kernel_spmd

    if "nc" not in _CACHE:
        _CACHE["nc"] = build_bass()
    nc = _CACHE["nc"]
    in_maps = prepare_inputs(**inputs)
    res = run_bass_kernel_spmd(nc, in_maps, core_ids=list(range(N_CORES)), trace=trace)
    out = (
        np.stack([res.results[c]["out"] for c in range(N_CORES)])
        .reshape(B, T, D)
        .astype(np.float32)
    )
    return out, res


def kernel(x, W_in, W_select, W_gate, W_out, W_delta, log_a):
    inputs = dict(
        x=x,
        W_in=W_in,
        W_select=W_select,
        W_gate=W_gate,
        W_out=W_out,
        W_delta=W_delta,
        log_a=log_a,
    )
    if not np.allclose(np.asarray(log_a, np.float32), 0.0):
        return _numpy_fallback(**inputs)
    out, _ = run_on_hw(inputs)
    return out
